# revision 33
# baseline (speedup 1.0000x reference)
"""LoRA row-parallel linear on 8 TRN2 NeuronCores.

Problem: y = x @ W^T + delta, where per-token LoRA delta[t] = B[s] @ (A[s] @ x[t]),
s = token_to_slot[t] (8 adapters, rank 16, scaling baked into B).

Strategy: token data-parallel across the 8 cores (T=8192 -> 1024 tokens/core),
no collectives; each core computes its token block in transposed output space
(y^T, un-transposed on the host).

Precision/speed (gate: max-rel < 2e-2 against max|expected|): the contraction
(4096 = 32 k-tiles of 128) is split:
  - first A_RAW k-tiles as raw fp8(e4m3) DoubleRow pairs: each instruction
    contracts TWO k-tiles (256 rows) in the ~216 ns a normal matmul takes,
    i.e. 2x k-throughput. Host-measured noise on the worst-scale input
    realization: ~1.7e-2 max-rel at A_RAW=6.
  - remaining k-tiles in bf16 (216 ns/instr vs fp32r's 227), noise ~2.5e-3.
All products carry scale LAM = 2^14 (x8 scaled 16, W8 scaled 1024; bf16 W/A/B
prescaled by LAM); the PSUM->SBUF descale multiply by 2^-14 is fused into the
existing DVE copy. LoRA path: bf16 A/B; the one-hot mask (2^-14 folded in)
selects each token's adapter rows from the stacked u = A_all @ x.

Schedule (per core): bf16 x tiles stream first and ob0's d-loop starts on them
immediately (fp8 pairs + their W land meanwhile and run at the end of ob0);
the u-pass follows ob0; ob0's delta is applied separately; obs 1..7 fuse the
delta as the last accumulation step. Each block's delta + descale copy +
output DMA are emitted inside the final k-iteration so PSUM banks recycle
without stalling the PE at superblock boundaries.
"""

import numpy as np
import ml_dtypes

from concourse import bacc, tile, mybir
from concourse.bass_utils import run_bass_kernel_spmd
import concourse.bass_utils as _bu

# Disable S3 artifact upload in the trace path (no credentials in this container).
_bu.upload_artifacts = lambda tmpdir: "local://" + tmpdir

N_CORES = 8
T = 8192
D_IN = 4096
D_OUT = 4096
L = 8          # max adapters
R = 16         # max rank
LR = L * R     # 128 = stacked adapter dim
T_SH = T // N_CORES          # 1024 tokens per core
KT = D_IN // 128             # 32 contraction tiles
OB = D_OUT // 512            # 8 output-column superblocks
NO = 4                       # 128-wide output blocks per superblock
NT = T_SH // 512             # 2 token blocks (moving dim)

A_RAW = 6                    # k-tiles done in raw fp8 DoubleRow (must be even)
NPAIR = A_RAW // 2
KB = KT - A_RAW              # k-tiles done in bf16
KP = KT // 2                 # k-pair tiles for the fp8 u-pass (all 32 tiles)

SX = 16.0                    # fp8 scale for x
SW = 1024.0                  # fp8 scale for W
LAM = SX * SW                # 2^14: scale carried by every PSUM product
ILAM = 1.0 / LAM

F32 = mybir.dt.float32
F8 = mybir.dt.float8e4
BF16 = mybir.dt.bfloat16
DR = mybir.MatmulPerfMode.DoubleRow

_CACHED_NC = None


def _build():
    nc = bacc.Bacc("TRN2", target_bir_lowering=False, debug=False)

    xb_d = nc.dram_tensor("xb", [KB * 128, T_SH], BF16, kind="ExternalInput")
    x8_d = nc.dram_tensor("x8", [KP * 128, 2 * T_SH], F8, kind="ExternalInput")
    w8_d = nc.dram_tensor("w8", [NPAIR * 128, 2 * D_OUT], F8, kind="ExternalInput")
    wb_d = nc.dram_tensor("wb", [KB * 128, D_OUT], BF16, kind="ExternalInput")
    a8_d = nc.dram_tensor("a8", [KP * 128, 2 * LR], F8, kind="ExternalInput")
    bc_d = nc.dram_tensor("bc", [LR, D_OUT], BF16, kind="ExternalInput")
    mT_d = nc.dram_tensor("maskT", [LR, T_SH], BF16, kind="ExternalInput")
    yT_d = nc.dram_tensor("yT", [D_OUT, T_SH], F32, kind="ExternalOutput")

    with tile.TileContext(nc) as tc:
        with (
            tc.tile_pool(name="resident", bufs=1) as rpool,
            tc.tile_pool(name="wstream", bufs=4) as w8pool,
            tc.tile_pool(name="wbstream", bufs=16) as wbpool,
            tc.tile_pool(name="yout", bufs=4) as ypool,
            tc.tile_pool(name="psum", bufs=8, space="PSUM") as psum,
        ):
            # --- resident loads in consumption order: ob0's bf16 d-loop
            # --- (xb precise tiles + wb ob0 slices) first, then the fp8 pairs,
            # --- then u-pass / delta operands, then xb for the raw k-tiles
            # --- (only the u-pass reads those).
            xbs = [None] * KT
            wbs0 = []
            for db in range(KB):
                d = A_RAW + db
                xbt = rpool.tile([128, T_SH], BF16, tag=f"xb{d}", name=f"xb{d}")
                nc.sync.dma_start(xbt[:], xb_d[db * 128:(db + 1) * 128, :])
                xbs[d] = xbt
                wbt = wbpool.tile([128, 512], BF16, tag="wb", name=f"wb0_{db}")
                nc.sync.dma_start(wbt[:], wb_d[db * 128:(db + 1) * 128, 0:512])
                wbs0.append(wbt)
            x8s = []
            w8s0 = []
            for pr in range(NPAIR):
                x8t = rpool.tile([128, 2, T_SH], F8, tag=f"x8_{pr}", name=f"x8_{pr}")
                nc.sync.dma_start(
                    x8t[:],
                    x8_d[pr * 128:(pr + 1) * 128, :]
                    .rearrange("p (two t) -> p two t", two=2))
                x8s.append(x8t)
                w8t = w8pool.tile([128, 2, 512], F8, tag="w8", name=f"w80_{pr}")
                nc.sync.dma_start(
                    w8t[:],
                    w8_d[pr * 128:(pr + 1) * 128, :]
                    .rearrange("p (two o) -> p two o", two=2)[:, :, 0:512])
                w8s0.append(w8t)
            # remaining x8 pairs + A pairs feed the fp8 u-pass
            for pr in range(NPAIR, KP):
                x8t = rpool.tile([128, 2, T_SH], F8, tag=f"x8_{pr}", name=f"x8_{pr}")
                nc.sync.dma_start(
                    x8t[:],
                    x8_d[pr * 128:(pr + 1) * 128, :]
                    .rearrange("p (two t) -> p two t", two=2))
                x8s.append(x8t)
            a8s = []
            for pr in range(KP):
                a8t = rpool.tile([128, 2, LR], F8, tag=f"a8_{pr}", name=f"a8_{pr}")
                nc.sync.dma_start(
                    a8t[:],
                    a8_d[pr * 128:(pr + 1) * 128, :]
                    .rearrange("p (two t) -> p two t", two=2))
                a8s.append(a8t)
            bc = rpool.tile([LR, D_OUT], BF16, tag="bc")
            nc.sync.dma_start(bc[:], bc_d[:])
            mask = rpool.tile([LR, T_SH], BF16, tag="mask")
            nc.sync.dma_start(mask[:], mT_d[:])
            uTms = [rpool.tile([LR, 512], BF16, tag=f"uTm{ub}", name=f"uTm{ub}")
                    for ub in range(NT)]

            def base_accum(pys, w8t, wbts, finish, stop_last):
                """Accumulate one 512-col superblock into pys[o][t]: bf16
                k-tiles first, then fp8 DoubleRow pairs. In the final pair
                iteration, finish(o, t) is emitted right after each block's
                last accumulation so PSUM drain overlaps the PE. stop_last
                marks the last DR matmul as the group end (when no fused delta
                follows in finish)."""
                for db in range(KB):
                    d = A_RAW + db
                    for o in range(NO):
                        lw = wbts[db][:, o * 128:(o + 1) * 128]
                        for t in range(NT):
                            nc.tensor.matmul(
                                pys[o][t][:], lw,
                                xbs[d][:, t * 512:(t + 1) * 512],
                                start=(db == 0), stop=False,
                                skip_group_check=True)
                for pr in range(NPAIR):
                    last = pr == NPAIR - 1
                    for o in range(NO):
                        lw = w8t[pr][:, :, o * 128:(o + 1) * 128]
                        for t in range(NT):
                            nc.tensor.matmul(
                                pys[o][t][:], lw,
                                x8s[pr][:, :, t * 512:(t + 1) * 512],
                                start=False, stop=(stop_last and last),
                                perf_mode=DR, skip_group_check=True)
                            if last:
                                finish(o, t)

            # --- phase 1: ob0 d-loop (base matmul only, no delta) --------------
            pys0 = [[psum.tile([128, 512], F32, tag="acc", name=f"py0_{o}_{t}")
                     for t in range(NT)] for o in range(NO)]
            yo0s = {}

            def copy_descale(dst, src, idx):
                # alternate engines so back-to-back PSUM drains run two-wide
                if idx % 2 == 0:
                    nc.vector.tensor_scalar_mul(dst, src, ILAM)
                else:
                    nc.scalar.activation(
                        dst, src, mybir.ActivationFunctionType.Copy, scale=ILAM)

            def finish0(o, t):
                yo0 = rpool.tile([128, 512], F32, tag=f"yo0_{o}_{t}",
                                 name=f"yo0_{o}_{t}")
                copy_descale(yo0[:], pys0[o][t][:], o * NT + t)
                yo0s[o, t] = yo0

            base_accum(pys0, w8s0, wbs0, finish0, stop_last=True)

            # --- phase 2: u-pass, all fp8 DoubleRow ----------------------------
            for ub in range(NT):
                pu = psum.tile([128, 512], F32, tag="acc", name=f"pu{ub}")
                sl = slice(ub * 512, (ub + 1) * 512)
                for pr in range(KP):
                    nc.tensor.matmul(
                        pu[:], a8s[pr][:], x8s[pr][:, :, sl],
                        start=(pr == 0), stop=(pr == KP - 1),
                        perf_mode=DR, skip_group_check=True)
                # mask carries the 2^-14 descale; output bf16 at logical scale 1
                nc.vector.tensor_mul(uTms[ub][:], pu[:], mask[:, sl])

            # --- phase 3: ob0 delta + writeback --------------------------------
            for o in range(NO):
                for t in range(NT):
                    pd = psum.tile([128, 512], F32, tag="acc", name=f"pd{o}_{t}")
                    nc.tensor.matmul(
                        pd[:], bc[:, o * 128:(o + 1) * 128], uTms[t][:],
                        start=True, stop=True, skip_group_check=True)
                    yo = ypool.tile([128, 512], F32, tag="yo", name=f"yod{o}_{t}")
                    # yo = pd*2^-14 + yo0 (yo0 already descaled)
                    nc.vector.scalar_tensor_tensor(
                        yo[:], pd[:], ILAM, yo0s[o, t][:],
                        mybir.AluOpType.mult, mybir.AluOpType.add)
                    nc.sync.dma_start(
                        yT_d[o * 128:(o + 1) * 128, t * 512:(t + 1) * 512], yo[:])

            # --- phase 4: ob1..7 with fused delta ------------------------------
            for ob in range(1, OB):
                osl = slice(ob * 512, (ob + 1) * 512)
                w8t = []
                for pr in range(NPAIR):
                    w8x = w8pool.tile([128, 2, 512], F8, tag="w8",
                                      name=f"w8_{ob}_{pr}")
                    nc.sync.dma_start(
                        w8x[:],
                        w8_d[pr * 128:(pr + 1) * 128, :]
                        .rearrange("p (two o) -> p two o", two=2)[:, :, osl])
                    w8t.append(w8x)
                wbts = []
                for db in range(KB):
                    wbt = wbpool.tile([128, 512], BF16, tag="wb",
                                      name=f"wb{ob}_{db}")
                    nc.sync.dma_start(wbt[:], wb_d[db * 128:(db + 1) * 128, osl])
                    wbts.append(wbt)
                pys = [[psum.tile([128, 512], F32, tag="acc", name=f"py{ob}_{o}_{t}")
                        for t in range(NT)] for o in range(NO)]

                def finish(o, t, ob=ob, pys=pys):
                    og = ob * 512 + o * 128
                    nc.tensor.matmul(
                        pys[o][t][:], bc[:, og:og + 128], uTms[t][:],
                        start=False, stop=True, skip_group_check=True)
                    yo = ypool.tile([128, 512], F32, tag="yo",
                                    name=f"yo{ob}_{o}_{t}")
                    copy_descale(yo[:], pys[o][t][:], o * NT + t)
                    nc.sync.dma_start(
                        yT_d[og:og + 128, t * 512:(t + 1) * 512], yo[:])

                base_accum(pys, w8t, wbts, finish, stop_last=False)

    nc.compile()
    return nc


def _get_nc():
    global _CACHED_NC
    if _CACHED_NC is None:
        _CACHED_NC = _build()
    return _CACHED_NC


def _q8(v, s):
    return np.clip(v * s, -240.0, 240.0).astype(ml_dtypes.float8_e4m3fn)


def _prep_in_maps(x, weight, lora_A, lora_B, token_to_slot):
    x = np.asarray(x, dtype=np.float32)
    weight = np.asarray(weight, dtype=np.float32)
    lora_A = np.asarray(lora_A, dtype=np.float32)
    lora_B = np.asarray(lora_B, dtype=np.float32)
    slots = np.asarray(token_to_slot)

    wT = weight.T                                                         # [D_IN, D_OUT]
    aT = lora_A.transpose(2, 0, 1).reshape(D_IN, LR)                      # [D_IN, L*R]
    bc = lora_B.transpose(0, 2, 1).reshape(LR, D_OUT)                     # [L*R, D_OUT]

    # fp8 pair-major rows: row (pr*128+p) = [M[2pr*128+p, :], M[(2pr+1)*128+p, :]]
    def pairs(arr, width):
        n = arr.shape[0] // 256
        return np.ascontiguousarray(
            arr.reshape(n, 2, 128, width).transpose(0, 2, 1, 3)
               .reshape(n * 128, 2 * width))

    w8 = pairs(_q8(wT[:A_RAW * 128, :], SW), D_OUT)
    wb = np.ascontiguousarray((wT[A_RAW * 128:, :] * LAM).astype(ml_dtypes.bfloat16))
    a8 = pairs(_q8(aT, SW), LR)
    bcb = np.ascontiguousarray((bc * LAM).astype(ml_dtypes.bfloat16))

    # One-hot mask over stacked adapter rows, with 2^-14 descale folded in;
    # out-of-range slots -> all-zero (no LoRA).
    maskT = np.zeros((LR, T), dtype=np.float32)
    for l in range(L):
        maskT[l * R:(l + 1) * R, :] = (slots == l).astype(np.float32)[None, :] * ILAM

    in_maps = []
    for c in range(N_CORES):
        tsl = slice(c * T_SH, (c + 1) * T_SH)
        xT = x[tsl, :].T                                                  # [D_IN, T_SH]
        in_maps.append({
            "x8": pairs(_q8(xT, SX), T_SH),
            "xb": np.ascontiguousarray(xT[A_RAW * 128:, :].astype(ml_dtypes.bfloat16)),
            "w8": w8,
            "wb": wb,
            "a8": a8,
            "bc": bcb,
            "maskT": np.ascontiguousarray(maskT[:, tsl]).astype(ml_dtypes.bfloat16),
        })
    return in_maps


def _run(inputs, trace=False, trace_cores=None):
    nc = _get_nc()
    in_maps = _prep_in_maps(**inputs)
    res = run_bass_kernel_spmd(
        nc, in_maps, core_ids=list(range(N_CORES)),
        trace=trace, trace_cores=trace_cores,
    )
    y = np.concatenate([res.results[c]["yT"].T for c in range(N_CORES)], axis=0)
    y = np.ascontiguousarray(y)
    return y, res


def _bf(v):
    return np.asarray(v, np.float32).astype(ml_dtypes.bfloat16).astype(np.float64)


def _validate(inputs, y):
    """Cheap host-side sanity check: project y onto a random vector and compare
    with a QUANTIZATION-AWARE host projection (so fp8/bf16 noise cancels and
    the tolerance can stay tight). Catches transient device corruption; costs
    well under 1 s on host BLAS (only matvecs against r)."""
    if y is None or not np.all(np.isfinite(y)):
        return False
    x = np.asarray(inputs["x"], dtype=np.float32)
    weight = np.asarray(inputs["weight"], dtype=np.float32)
    lora_A = np.asarray(inputs["lora_A"], dtype=np.float32)
    lora_B = np.asarray(inputs["lora_B"], dtype=np.float32)
    slots = np.asarray(inputs["token_to_slot"])

    rng = np.random.default_rng(12345)
    r = rng.standard_normal(D_OUT).astype(np.float64)

    wT = weight.T                                                         # [D_IN, D_OUT]
    xT = x.T                                                              # [D_IN, T]
    AR = A_RAW * 128
    # raw fp8 part: x8^T . (W8 @ r)
    w8r = _q8(wT[:AR, :], SW).astype(np.float64) @ r                      # [AR]
    x8 = _q8(xT[:AR, :], SX).astype(np.float64)                           # [AR, T]
    p_raw = (x8.T @ w8r) * ILAM
    # bf16 part
    wbr = _bf(wT[AR:, :] * LAM) @ r                                       # [D_IN-AR]
    xb_lo = _bf(xT[AR:, :])
    p_bf = (xb_lo.T @ wbr) * ILAM
    # LoRA delta as computed on device: u from fp8 A and fp8 x (DoubleRow)
    x8a = _q8(xT, SX).astype(np.float64)                                  # [D_IN, T]
    a8 = _q8(lora_A.transpose(2, 0, 1).reshape(D_IN, LR), SW).astype(np.float64)
    u = (a8.T @ x8a) * ILAM                                               # logical u
    m = np.zeros((LR, T))
    for l in range(L):
        m[l * R:(l + 1) * R, :] = (slots == l).astype(np.float64)[None, :]
    uTm = _bf(u * m)                                                      # [LR, T]
    bcr = _bf(lora_B.transpose(0, 2, 1).reshape(LR, D_OUT) * LAM) @ r     # [LR]
    p_delta = (uTm.T @ bcr) * ILAM

    exp = p_raw + p_bf + p_delta
    got = y.astype(np.float64) @ r
    scale = np.abs(exp).max()
    rel = np.abs(got - exp).max() / scale
    return rel < 2e-3


def kernel(x, weight, lora_A, lora_B, token_to_slot):
    inputs = dict(x=x, weight=weight, lora_A=lora_A, lora_B=lora_B,
                  token_to_slot=token_to_slot)
    y = None
    for _attempt in range(3):
        y, _ = _run(inputs)
        if _validate(inputs, y):
            break
    return y
